# revision 1
# baseline (speedup 1.0000x reference)
import numpy as np

# nn_NearestNeighbours: batch [8,512,512] f32, emb [50000,512] f32,
# output argmin indices [8,512] int32. Vocab-sharded across 8 cores.
# Screen: fp8e4m3 DoubleRow GEMM -> f16(score+128) packed with u16 local idx
# -> pairwise packed-word max 6400->3200 -> DVE top-8 -> exact f64 rescore.
B, S, E, V = 8, 512, 512, 50000
R = B * S              # 4096 token rows
NC = 8                 # cores
VS = V // NC           # 6250 vocab rows per core
VSP = 6400             # padded to 6*1024 + 256
GEN_LAST = VS - 6144   # 106 genuine cols in last chunk
KT = E // 128          # 4 k-subtiles
MT = R // 128          # 32 m-tiles
HALF = VSP // 2        # 3200 pre-reduce width

# HW-legal engine split: Pool cannot read PSUM nor run tensor ops on HW,
# so ACT does ALL PSUM->SBUF evictions (1024-wide chunks) and DVE does the
# straight packed-word max8 over the full 6400-word half.
ACT_WORDS = VSP

_CACHE = {}


def _build():
    import concourse.bacc as bacc
    import concourse.mybir as mybir
    from concourse.tile import TileContext

    dtf = mybir.dt.float32
    dt8 = mybir.dt.float8e4
    dth = mybir.dt.float16
    dtu16 = mybir.dt.uint16
    DR = mybir.MatmulPerfMode.DoubleRow

    nc = bacc.Bacc("TRN2", target_bir_lowering=False, debug=False)
    bT_ap = nc.dram_tensor("bT", [E, R], dt8, kind="ExternalInput").ap()
    embT_ap = nc.dram_tensor("embT", [E, VSP], dt8, kind="ExternalInput").ap()
    out_ap = nc.dram_tensor("out", [R, 8], dtf, kind="ExternalOutput").ap()

    with TileContext(nc) as tc:
        with tc.sbuf_pool(name="emb", bufs=1) as embp, \
             tc.sbuf_pool(name="pk", bufs=1) as pkp, \
             tc.sbuf_pool(name="bt", bufs=2) as btp, \
             tc.sbuf_pool(name="val", bufs=2) as valp, \
             tc.psum_pool(name="psA", bufs=3) as psA, \
             tc.psum_pool(name="psB", bufs=2) as psB:
            pk = pkp.tile([128, 4 * VSP], dtu16, name="pk")
            bias = embp.tile([128, 1], dtf, name="bias")
            nc.vector.memset(bias[:], 128.0)
            for h in range(2):
                nc.gpsimd.iota(pk[:, 2 * VSP * h:2 * VSP * (h + 1):2],
                               pattern=[[1, VSP]], base=0, channel_multiplier=0)
            gt = btp.tile([128, KT, 512], dt8)
            for k in range(KT):
                nc.scalar.dma_start(gt[:, k:k + 1, :],
                                    bT_ap[128 * k:128 * (k + 1), 0:512])
            emb8 = embp.tile([128, KT, VSP], dt8, name="emb8")
            off = 0
            for w in [1024] * 6 + [256]:
                eng = nc.scalar if off >= 5120 else nc.sync
                for k in range(KT):
                    eng.dma_start(
                        emb8[:, k:k + 1, off:off + w],
                        embT_ap[128 * k:128 * (k + 1), off:off + w],
                    )
                off += w
            pk16 = pk[:].bitcast(dth)
            pk32 = pk[:].bitcast(dtf)

            def evict(eng, dst, src):
                if eng == "A":
                    nc.scalar.add(dst, src, bias[:])
                elif eng == "P":
                    nc.gpsimd.tensor_scalar_add(dst, src, 128.0)
                else:
                    nc.vector.tensor_scalar_add(dst, src, 128.0)

            for g in range(MT // 4):
                cur = gt
                if g + 1 < MT // 4:
                    gt = btp.tile([128, KT, 512], dt8)
                    for k in range(KT):
                        nc.sync.dma_start(
                            gt[:, k:k + 1, :],
                            bT_ap[128 * k:128 * (k + 1), 512 * (g + 1):512 * (g + 2)],
                        )
                for mm in range(4):
                    m = g * 4 + mm
                    h16 = (m % 2) * 2 * VSP
                    base = (m % 2) * VSP

                    def evict(off, wv, pt):
                        a1 = min(off + wv, ACT_WORDS)
                        if a1 > off:
                            nc.scalar.add(
                                pk16[:, h16 + 2 * off + 1:h16 + 2 * a1:2],
                                pt[:, 0:a1 - off], bias[:])
                        d0 = max(off, ACT_WORDS)
                        if off + wv > d0:
                            nc.vector.tensor_scalar_add(
                                pk16[:, h16 + 2 * d0 + 1:h16 + 2 * (off + wv):2],
                                pt[:, d0 - off:wv], 128.0)

                    for n in range(6):
                        off = n * 1024
                        pt = psA.tile([128, 1024], dtf)
                        for h in range(2):
                            for p in range(2):
                                nc.tensor.matmul(
                                    pt[:, 512 * h:512 * (h + 1)],
                                    cur[:, 2 * p:2 * p + 2, 128 * mm:128 * mm + 128],
                                    emb8[:, 2 * p:2 * p + 2,
                                         off + 512 * h:off + 512 * h + 512],
                                    start=(p == 0),
                                    stop=(p == 1),
                                    perf_mode=DR,
                                )
                        evict(off, 1024, pt)
                    off = 6144
                    pt = psB.tile([128, 256], dtf)
                    for p in range(2):
                        nc.tensor.matmul(
                            pt[:],
                            cur[:, 2 * p:2 * p + 2, 128 * mm:128 * mm + 128],
                            emb8[:, 2 * p:2 * p + 2, off:off + 256],
                            start=(p == 0),
                            stop=(p == 1),
                            perf_mode=DR,
                        )
                    evict(off, GEN_LAST, pt)
                    vt = valp.tile([128, 8], dtf)
                    nc.vector.max(vt[:], pk32[:, base:base + VS])
                    nc.sync.dma_start(out_ap[128 * m:128 * (m + 1), :], vt[:])
    nc.compile()
    return nc


def _run(batch: np.ndarray, emb: np.ndarray, trace: bool = False):
    import ml_dtypes
    from concourse import bass_utils

    if "nc" not in _CACHE:
        _CACHE["nc"] = _build()
    nc = _CACHE["nc"]
    f8 = ml_dtypes.float8_e4m3

    b = np.ascontiguousarray(batch.reshape(R, E).astype(np.float32))
    bT8 = np.ascontiguousarray(b.T).astype(f8)
    embT8 = emb.T.astype(f8)
    in_maps = []
    for c in range(NC):
        shardT = np.zeros((E, VSP), f8)
        shardT[:, :VS] = embT8[:, c * VS:(c + 1) * VS]
        in_maps.append({"bT": bT8, "embT": shardT})

    res = bass_utils.run_bass_kernel_spmd(
        nc, in_maps, core_ids=list(range(NC)), trace=trace
    )

    # unpack candidates: low 16 bits of each packed word = local vocab idx
    G = np.empty((R, NC * 8), np.int64)
    for c in range(NC):
        raw = np.ascontiguousarray(res.results[c]["out"]).view(np.uint32)
        idx = (raw & 0xFFFF).astype(np.int64)
        G[:, c * 8:(c + 1) * 8] = c * VS + np.minimum(idx, VS - 1)

    # exact rescore in f64 on normalized embeddings
    emb64 = emb.astype(np.float64)
    inv = 1.0 / np.sqrt((emb64 * emb64).sum(axis=1) + 1e-12)
    b64 = b.astype(np.float64)
    best = np.empty(R, np.int64)
    CH = 256
    for r0 in range(0, R, CH):
        r1 = min(r0 + CH, R)
        g = G[r0:r1]
        ce = emb64[g] * inv[g][:, :, None]          # [ch,64,512]
        s = np.matmul(ce, b64[r0:r1, :, None])[:, :, 0]
        am = np.argmax(s, axis=1)
        best[r0:r1] = g[np.arange(r1 - r0), am]

    return best.astype(np.int32).reshape(B, S), res


def kernel(batch: np.ndarray, emb: np.ndarray) -> np.ndarray:
    out, _ = _run(batch, emb, trace=False)
    return out



# revision 2
# speedup vs baseline: 1.2135x; 1.2135x over previous
import numpy as np

# nn_NearestNeighbours: batch [8,512,512] f32, emb [50000,512] f32,
# output argmin cosine-distance indices [8,512] int32.
#
# Strategy: vocab-sharded fp8 DoubleRow screen GEMM over 8 cores
# (6144 vocab cols per core, 49152 total; the 848-col tail is scored
# exactly on the host). Per 128-row m-tile each core evicts raw-dot
# scores PSUM->SBUF as contiguous f16 (split between ACT and DVE),
# folds 6144 -> 3072 -> 1536 with f16 tensor-max (2x DVE mode), and
# DMAs the folded cell table to HBM. The host finds cells within a
# screen margin of the row-global best, expands each cell to its 4
# source columns, and exact-rescores candidates in f64 on normalized
# embeddings (plus the tail block) to pick the final argmax.
B, S, E, V = 8, 512, 512, 50000
R = B * S              # 4096 token rows
NC = 8                 # cores
VS = 6144              # vocab cols per core on device
VTAIL = NC * VS        # 49152; [VTAIL, V) scored on host
KT = E // 128          # 4 k-subtiles
MT = R // 128          # 32 m-tiles
CH = 2048              # psum chunk width (3 chunks per m-tile)
DS = 768               # leading cols of chunk 2 evicted by DVE (rest ACT)
FW = 1536              # folded cell-table width per m-tile
MARGIN = 16.0          # screen-score pruning margin (max observed gap 12.3)

_CACHE = {}


def _build():
    import concourse.bacc as bacc
    import concourse.mybir as mybir
    from concourse.tile import TileContext

    dtf = mybir.dt.float32
    dt8 = mybir.dt.float8e4
    dth = mybir.dt.float16
    DR = mybir.MatmulPerfMode.DoubleRow
    Copy = mybir.ActivationFunctionType.Copy

    nc = bacc.Bacc("TRN2", target_bir_lowering=False, debug=False)
    bT_ap = nc.dram_tensor("bT", [E, R], dt8, kind="ExternalInput").ap()
    embT_ap = nc.dram_tensor("embT", [E, VS], dt8, kind="ExternalInput").ap()
    outF_ap = nc.dram_tensor("outF", [R, FW], dth, kind="ExternalOutput").ap()

    with TileContext(nc) as tc:
        with tc.sbuf_pool(name="emb", bufs=1) as embp, \
             tc.sbuf_pool(name="bt", bufs=2) as btp, \
             tc.sbuf_pool(name="wk", bufs=1) as wkp, \
             tc.psum_pool(name="ps", bufs=2) as ps:
            gt = btp.tile([128, KT, 512], dt8)
            for k in range(KT):
                nc.scalar.dma_start(gt[:, k:k + 1, :],
                                    bT_ap[128 * k:128 * (k + 1), 0:512])
            emb8 = embp.tile([128, KT, VS], dt8, name="emb8")
            for off in range(0, VS, 1024):
                eng = nc.scalar if off >= 4096 else nc.sync
                for k in range(KT):
                    eng.dma_start(
                        emb8[:, k:k + 1, off:off + 1024],
                        embT_ap[128 * k:128 * (k + 1), off:off + 1024],
                    )
            h = wkp.tile([128, 2, VS], dth, name="h")
            t1 = wkp.tile([128, 2, VS // 2], dth, name="t1")
            fc = wkp.tile([128, 2, FW], dth, name="fc")

            for g in range(MT // 4):
                cur = gt
                if g + 1 < MT // 4:
                    gt = btp.tile([128, KT, 512], dt8)
                    for k in range(KT):
                        nc.sync.dma_start(
                            gt[:, k:k + 1, :],
                            bT_ap[128 * k:128 * (k + 1), 512 * (g + 1):512 * (g + 2)],
                        )
                for mm in range(4):
                    m = g * 4 + mm
                    hf = m % 2
                    for c in range(3):
                        pt = ps.tile([128, CH], dtf)
                        for p in range(2):
                            for k in range(4):
                                nc.tensor.matmul(
                                    pt[:, 512 * k:512 * (k + 1)],
                                    cur[:, 2 * p:2 * p + 2, 128 * mm:128 * mm + 128],
                                    emb8[:, 2 * p:2 * p + 2,
                                         CH * c + 512 * k:CH * c + 512 * (k + 1)],
                                    start=(p == 0),
                                    stop=(p == 1),
                                    perf_mode=DR,
                                )
                        if c < 2:
                            nc.scalar.activation(
                                h[:, hf, CH * c:CH * (c + 1)], pt[:], Copy)
                        else:
                            nc.vector.tensor_copy(
                                h[:, hf, CH * c:CH * c + DS], pt[:, 0:DS])
                            nc.scalar.activation(
                                h[:, hf, CH * c + DS:CH * (c + 1)],
                                pt[:, DS:CH], Copy)
                    half = VS // 2
                    nc.vector.tensor_max(
                        t1[:, hf, :], h[:, hf, 0:half], h[:, hf, half:VS])
                    nc.vector.tensor_max(
                        fc[:, hf, :], t1[:, hf, 0:FW], t1[:, hf, FW:half])
                    nc.gpsimd.dma_start(
                        outF_ap[128 * m:128 * (m + 1), :], fc[:, hf, :])
    nc.compile()
    return nc


def _run(batch: np.ndarray, emb: np.ndarray, trace: bool = False):
    import ml_dtypes
    from concourse import bass_utils

    if "nc" not in _CACHE:
        _CACHE["nc"] = _build()
    nc = _CACHE["nc"]
    f8 = ml_dtypes.float8_e4m3

    b = np.ascontiguousarray(batch.reshape(R, E).astype(np.float32))
    bT8 = np.ascontiguousarray(b.T).astype(f8)
    embT8 = emb[:VTAIL].T.astype(f8)
    in_maps = []
    for c in range(NC):
        in_maps.append({
            "bT": bT8,
            "embT": np.ascontiguousarray(embT8[:, c * VS:(c + 1) * VS]),
        })

    res = bass_utils.run_bass_kernel_spmd(
        nc, in_maps, core_ids=list(range(NC)), trace=trace
    )

    # [R, NC, FW] screen cell table (cell j = max of 4 source columns)
    Fall = np.stack(
        [np.asarray(res.results[c]["outF"]) for c in range(NC)], axis=1
    ).astype(np.float32)
    gbest = Fall.max(axis=(1, 2))
    rows, cores, cells = np.nonzero(Fall >= (gbest - MARGIN)[:, None, None])

    # expand cells to their 4 source columns -> global vocab candidates
    cand = (cores * VS)[:, None] + cells[:, None] + \
        np.array([0, FW, 2 * FW, 3 * FW])[None, :]
    cand = cand.reshape(-1)
    crow = np.repeat(rows, 4)

    # exact rescore in f64 on normalized embeddings
    emb64 = emb.astype(np.float64)
    inv = 1.0 / np.sqrt((emb64 * emb64).sum(axis=1) + 1e-12)
    b64 = b.astype(np.float64)
    s = np.einsum("ij,ij->i", emb64[cand] * inv[cand][:, None], b64[crow])

    best_s = np.full(R, -np.inf)
    best_i = np.zeros(R, np.int64)
    order = np.argsort(crow, kind="stable")
    crow_s, cand_s, s_s = crow[order], cand[order], s[order]
    np.maximum.at(best_s, crow_s, s_s)
    hit = s_s == best_s[crow_s]
    best_i[crow_s[hit]] = cand_s[hit]
    # (ties: last writer wins; resolved below against exact tail anyway)

    # exact tail block [VTAIL, V)
    tail = (emb64[VTAIL:] * inv[VTAIL:, None]) @ b64.T   # [848, R]
    t_best = tail.max(axis=0)
    t_idx = VTAIL + tail.argmax(axis=0)
    use_tail = t_best > best_s
    best_i[use_tail] = t_idx[use_tail]

    return best_i.astype(np.int32).reshape(B, S), res


def kernel(batch: np.ndarray, emb: np.ndarray) -> np.ndarray:
    out, _ = _run(batch, emb, trace=False)
    return out
